# revision 11
# baseline (speedup 1.0000x reference)
"""EGNN graph-conv (DGL EnGraphConv style) Trainium2 kernel, 8 NeuronCores.

Sharding: edges partitioned by dst node-range (edge/dst-parallel) — each core
owns 6250 nodes and all edges pointing into them, so the segment-sum needs no
cross-core reduction (outputs concatenate).

Device pipeline (per core, feature-major: features on partitions, edges on
the free axis):
  - node features gathered per edge endpoint with the SWDGE transpose
    dma_gather (bf16, 256B rows) — src from the replicated global table in two
    int16-index windows, dst from the core-local table,
  - edge MLP / attention gate / coord head as chained PE matmuls (f32 PSUM),
  - segment-sum scatter as a per-128-edge-group matmul against an on-chip
    0/1 selection matrix S (iota==dstrel), accumulated per 64-node bin in
    PSUM, evacuated to an SBUF accumulator,
  - node MLP + residual in f32, outputs written feature-major and
    transposed/concatenated on the host.

Host-side prep is index/layout work only (sort edges, build CSR-like bins,
int16 gather indices, per-edge geometric features radial/coord_diff, degree
counts) — all model FLOPs (MLPs, gate, scatter sums, residuals) run on
device.
"""
import numpy as np
import ml_dtypes

N = 50000
E = 800000
D = 128
DE = 32
NCORES = 8
NLOC = N // NCORES            # 6250
BINW = 64
NBINS = (NLOC + BINW - 1) // BINW   # 98
NPAD = NBINS * BINW           # 6272
WIN = 32768
BASE1 = N - WIN               # 17232
G = 128                       # scatter group (edges)
CE = 512                      # matmul chunk (edges)

bf16 = ml_dtypes.bfloat16
f32 = np.float32


def _bf(x):
    return np.asarray(x, dtype=bf16)


def _prep(inputs):
    src = np.asarray(inputs["src"])
    dst = np.asarray(inputs["dst"])
    nf = np.asarray(inputs["node_feats"], f32)
    coords = np.asarray(inputs["coords"], f32)
    ef = np.asarray(inputs["edge_feats"], f32)

    core_of = dst // NLOC
    deg = np.bincount(dst, minlength=N).astype(f32)

    per_core = []
    cnt = np.zeros((NCORES, 2, NBINS), np.int64)
    for k in range(NCORES):
        sel = np.nonzero(core_of == k)[0]
        s, d = src[sel], dst[sel]
        dloc = d - k * NLOC
        b = dloc // BINW
        w = (s >= WIN).astype(np.int64)
        order = np.lexsort((s, b, w))
        per_core.append((sel[order], s[order], dloc[order], b[order], w[order]))
        np.add.at(cnt[k], (w, b), 1)

    Gwb = np.maximum(1, -(-cnt.max(axis=0) // G))       # [2, NBINS]
    for wv in range(2):
        Gwb[wv, NBINS - 1] += (-Gwb[wv].sum()) % (CE // G)
    ngroups = int(Gwb.sum())
    EC = ngroups * G
    group_bin, group_win = [], []
    for wv in range(2):
        for bv in range(NBINS):
            group_bin += [bv] * int(Gwb[wv, bv])
            group_win += [wv] * int(Gwb[wv, bv])
    sw0 = int(Gwb[0].sum()) * G

    cores = []
    for k in range(NCORES):
        eidx, s, dloc, b, w = per_core[k]
        idx_src = np.zeros(EC, np.int16)
        idx_dst = np.zeros(EC, np.int16)
        efx = np.zeros((EC, DE + 1), f32)
        cd3 = np.zeros((EC, 3), f32)
        dstrel = np.full(EC, -1.0, f32)
        pos = 0
        ptr = 0
        for wv in range(2):
            for bv in range(NBINS):
                n = int(cnt[k, wv, bv])
                seg = slice(ptr, ptr + n)
                o = slice(pos, pos + n)
                e_ids = eidx[seg]
                idx_src[o] = (s[seg] - (BASE1 if wv else 0)).astype(np.int16)
                idx_dst[o] = dloc[seg].astype(np.int16)
                efx[o, :DE] = ef[e_ids]
                diff = coords[src[e_ids]] - coords[dst[e_ids]]
                w_ic = 1.0 / np.maximum(deg[dst[e_ids]], 1.0)
                cd3[o] = diff * w_ic[:, None]
                efx[o, DE] = (diff * diff).sum(-1)
                dstrel[o] = (dloc[seg] - bv * BINW).astype(f32)
                ptr += n
                pos += int(Gwb[wv, bv]) * G

        nf16_loc = np.zeros((NPAD, D), bf16)
        nf16_loc[:NLOC] = _bf(nf[k * NLOC:(k + 1) * NLOC])
        nfT_loc = np.zeros((D, NPAD), f32)
        nfT_loc[:, :NLOC] = nf[k * NLOC:(k + 1) * NLOC].T
        coordsT_loc = np.zeros((3, NPAD), f32)
        coordsT_loc[:, :NLOC] = coords[k * NLOC:(k + 1) * NLOC].T
        cores.append(dict(
            idx_src=np.tile(idx_src.reshape(-1, 16).T, (8, 1)).copy(),
            idx_dst=np.tile(idx_dst.reshape(-1, 16).T, (8, 1)).copy(),
            efT=_bf(efx.T).copy(),
            cd3=np.ascontiguousarray(
                cd3.reshape(ngroups, G, 3).transpose(1, 0, 2).reshape(G, -1)),
            dstrel=np.ascontiguousarray(dstrel.reshape(ngroups, G).T),
            nf16_loc=nf16_loc, nfT_loc=nfT_loc,
            coordsT_loc=coordsT_loc,
        ))

    shared = dict(
        nf16_glob=_bf(nf),
        iota=np.broadcast_to(np.arange(BINW, dtype=f32), (G, BINW)).copy(),
        ident_ext=np.concatenate(
            [np.eye(D, dtype=f32), np.asarray(inputs["W_a"], f32)], 1).astype(bf16),
        W_e1a=_bf(inputs["W_e1"][:D]), W_e1b=_bf(inputs["W_e1"][D:2 * D]),
        W_e1c=_bf(np.concatenate(
            [inputs["W_e1"][2 * D + 1:], inputs["W_e1"][2 * D:2 * D + 1]], 0)),
        W_e2=_bf(inputs["W_e2"]), W_c1=_bf(inputs["W_c1"]), W_c=_bf(inputs["W_c"]),
        W_n1a=np.asarray(inputs["W_n1"][:D], f32),
        W_n1b=np.asarray(inputs["W_n1"][D:], f32),
        W_n2=np.asarray(inputs["W_n2"], f32),
        b_e1=np.asarray(inputs["b_e1"], f32).reshape(D, 1),
        b_e2=np.asarray(inputs["b_e2"], f32).reshape(D, 1),
        b_c1=np.asarray(inputs["b_c1"], f32).reshape(D, 1),
        b_n1=np.asarray(inputs["b_n1"], f32).reshape(D, 1),
        b_n2=np.asarray(inputs["b_n2"], f32).reshape(D, 1),
    )
    meta = dict(EC=EC, ngroups=ngroups, group_bin=group_bin,
                group_win=group_win, sw0=sw0,
                b_a=float(np.asarray(inputs["b_a"]).reshape(())))
    return shared, cores, meta


def _build_nc(meta):
    import concourse.bacc as bacc
    import concourse.mybir as mybir
    import concourse.tile as tile
    from concourse import library_config

    EC, ngroups = meta["EC"], meta["ngroups"]
    gb, gw, sw0 = meta["group_bin"], meta["group_win"], meta["sw0"]
    b_a = meta["b_a"]
    dt = mybir.dt
    AF = mybir.ActivationFunctionType
    OP = mybir.AluOpType

    nc = bacc.Bacc("TRN2", target_bir_lowering=False)

    # dram I/O
    din = {}
    def I(name, shape, dtype):
        din[name] = nc.dram_tensor(name, shape, dtype, kind="ExternalInput")
        return din[name]

    I("idx_src", [128, EC // 16], dt.int16)
    I("idx_dst", [128, EC // 16], dt.int16)
    I("efT", [DE + 1, EC], dt.bfloat16)
    I("cd3", [G, ngroups * 3], dt.float32)
    I("dstrel", [G, ngroups], dt.float32)
    I("nf16_loc", [NPAD, D], dt.bfloat16)
    I("nfT_loc", [D, NPAD], dt.float32)
    I("coordsT_loc", [3, NPAD], dt.float32)
    I("nf16_glob", [N, D], dt.bfloat16)
    I("iota", [G, BINW], dt.float32)
    I("ident_ext", [D, D + 1], dt.bfloat16)
    I("W_e1a", [D, D], dt.bfloat16)
    I("W_e1b", [D, D], dt.bfloat16)
    I("W_e1c", [DE + 1, D], dt.bfloat16)
    I("W_e2", [D, D], dt.bfloat16)
    I("W_c1", [D, D], dt.bfloat16)
    I("W_c", [D, 1], dt.bfloat16)
    I("W_n1a", [D, D], dt.float32)
    I("W_n1b", [D, D], dt.float32)
    I("W_n2", [D, D], dt.float32)
    for bname in ["b_e1", "b_e2", "b_c1", "b_n1", "b_n2"]:
        I(bname, [D, 1], dt.float32)

    houtT = nc.dram_tensor("houtT", [D, NPAD], dt.float32, kind="ExternalOutput")
    coutT = nc.dram_tensor("coutT", [3, NPAD], dt.float32, kind="ExternalOutput")

    nchunks = EC // CE

    with tile.TileContext(nc) as tc:
        nc.gpsimd.load_library(library_config.mlp)
        with (
            tc.tile_pool(name="const", bufs=1) as cpool,
            tc.tile_pool(name="gath", bufs=3) as gpool,
            tc.tile_pool(name="work", bufs=3) as wpool,
            tc.tile_pool(name="grp", bufs=4) as grpool,
            tc.tile_pool(name="acc", bufs=1) as apool,
            tc.tile_pool(name="pbig", bufs=3, space="PSUM") as ps_big,
            tc.tile_pool(name="ptg", bufs=2, space="PSUM") as ps_tg,
            tc.tile_pool(name="psc", bufs=1, space="PSUM") as ps_sc,
        ):
            # ---- constants to SBUF ----
            def load_const(name, shape, dtype):
                t = cpool.tile(shape, dtype, tag=f"c_{name}")
                nc.sync.dma_start(t[:], din[name][:])
                return t

            idx_src = load_const("idx_src", [128, EC // 16], dt.int16)
            idx_dst = load_const("idx_dst", [128, EC // 16], dt.int16)
            cd3 = load_const("cd3", [G, ngroups * 3], dt.float32)
            dstrel = load_const("dstrel", [G, ngroups], dt.float32)
            nfT_loc = load_const("nfT_loc", [D, NPAD], dt.float32)
            coordsT_loc = load_const("coordsT_loc", [3, NPAD], dt.float32)
            iota = load_const("iota", [G, BINW], dt.float32)
            ident_ext = load_const("ident_ext", [D, D + 1], dt.bfloat16)
            W_e1a = load_const("W_e1a", [D, D], dt.bfloat16)
            W_e1b = load_const("W_e1b", [D, D], dt.bfloat16)
            W_e1c = load_const("W_e1c", [DE + 1, D], dt.bfloat16)
            W_e2 = load_const("W_e2", [D, D], dt.bfloat16)
            W_c1 = load_const("W_c1", [D, D], dt.bfloat16)
            W_c = load_const("W_c", [D, 1], dt.bfloat16)
            W_n1a = load_const("W_n1a", [D, D], dt.float32)
            W_n1b = load_const("W_n1b", [D, D], dt.float32)
            W_n2 = load_const("W_n2", [D, D], dt.float32)
            b_e1 = load_const("b_e1", [D, 1], dt.float32)
            b_e2 = load_const("b_e2", [D, 1], dt.float32)
            b_c1 = load_const("b_c1", [D, 1], dt.float32)
            b_n1 = load_const("b_n1", [D, 1], dt.float32)
            b_n2 = load_const("b_n2", [D, 1], dt.float32)

            magg = apool.tile([D, NPAD], dt.float32)     # segment-sum accum
            cagg = apool.tile([3, NPAD], dt.float32)

            src_windows = [din["nf16_glob"][0:WIN, :],
                           din["nf16_glob"][BASE1:BASE1 + WIN, :]]

            psa = psb = None
            for c in range(nchunks):
                e0 = c * CE
                w = gw[c * (CE // G)]
                assert all(gw[c * (CE // G) + j] == w for j in range(CE // G))

                src_t = gpool.tile([128, 1, CE], dt.bfloat16, tag="src")
                nc.gpsimd.dma_gather(
                    src_t[:], src_windows[w], idx_src[:, e0 // 16:(e0 + CE) // 16],
                    CE, CE, D, transpose=True, single_packet=False)
                dst_t = gpool.tile([128, 1, CE], dt.bfloat16, tag="dst")
                nc.gpsimd.dma_gather(
                    dst_t[:], din["nf16_loc"][:], idx_dst[:, e0 // 16:(e0 + CE) // 16],
                    CE, CE, D, transpose=True, single_packet=False)
                ef_t = gpool.tile([DE + 1, CE], dt.bfloat16, tag="ef")
                nc.sync.dma_start(ef_t[:], din["efT"][:, e0:e0 + CE])

                m1_ps = ps_big.tile([D, CE], dt.float32, tag="big")
                nc.tensor.matmul(m1_ps[:], W_e1a[:], src_t[:, 0, :],
                                 start=True, stop=False)
                nc.tensor.matmul(m1_ps[:], W_e1b[:], dst_t[:, 0, :],
                                 start=False, stop=False)
                nc.tensor.matmul(m1_ps[:], W_e1c[:], ef_t[:],
                                 start=False, stop=True)
                m1s = wpool.tile([D, CE], dt.bfloat16, tag="m1s")
                nc.scalar.activation(m1s[:], m1_ps[:], AF.Silu, bias=b_e1[:])

                m2_ps = ps_big.tile([D, CE], dt.float32, tag="big")
                nc.tensor.matmul(m2_ps[:], W_e2[:], m1s[:], start=True, stop=True)
                mT = wpool.tile([D, CE], dt.bfloat16, tag="mT")
                nc.scalar.activation(mT[:], m2_ps[:], AF.Silu, bias=b_e2[:])

                c1_ps = ps_big.tile([D, CE], dt.float32, tag="big")
                nc.tensor.matmul(c1_ps[:], W_c1[:], mT[:], start=True, stop=True)
                c1T = wpool.tile([D, CE], dt.bfloat16, tag="c1T")
                nc.scalar.activation(c1T[:], c1_ps[:], AF.Silu, bias=b_c1[:])

                for j in range(CE // G):
                    g = c * (CE // G) + j
                    gs = slice(j * G, (j + 1) * G)
                    b0 = gb[g] * BINW

                    tg_ps = ps_tg.tile([G, D + 1], dt.float32, tag="tg")
                    nc.tensor.matmul(tg_ps[:], mT[:, gs], ident_ext[:],
                                     start=True, stop=True)
                    siga = grpool.tile([G, 1], dt.float32, tag="siga")
                    nc.scalar.activation(siga[:], tg_ps[:, D:D + 1],
                                         AF.Sigmoid, bias=b_a)
                    c_ps = ps_sc.tile([G, 1], dt.float32, tag="cps")
                    nc.tensor.matmul(c_ps[:], c1T[:, gs], W_c[:],
                                     start=True, stop=True)
                    c_sb = grpool.tile([G, 1], dt.float32, tag="csb")
                    nc.vector.tensor_copy(c_sb[:], c_ps[:])

                    pm = grpool.tile([G, D], dt.bfloat16, tag="pm")
                    nc.vector.tensor_scalar_mul(pm[:], tg_ps[:, 0:D], siga[:])
                    pay3 = grpool.tile([G, 3], dt.bfloat16, tag="pay3")
                    nc.vector.tensor_scalar_mul(
                        pay3[:], cd3[:, g * 3:(g + 1) * 3], c_sb[:])
                    S = grpool.tile([G, BINW], dt.bfloat16, tag="S")
                    nc.vector.tensor_scalar(
                        S[:], iota[:], dstrel[:, g:g + 1], None, op0=OP.is_equal)

                    first = (g == 0) or (gb[g - 1] != gb[g]) or (gw[g - 1] != gw[g])
                    last = (g == ngroups - 1) or (gb[g + 1] != gb[g]) \
                        or (gw[g + 1] != gw[g])
                    if first:
                        psa = ps_sc.tile([D, BINW], dt.float32, tag="psa")
                        psb = ps_sc.tile([3, BINW], dt.float32, tag="psb")
                    nc.tensor.matmul(psa[:], pm[:], S[:], start=first, stop=last)
                    nc.tensor.matmul(psb[:], pay3[:], S[:], start=first, stop=last)
                    if last:
                        dsl = slice(b0, b0 + BINW)
                        if gw[g] == 0:
                            nc.vector.tensor_copy(magg[:, dsl], psa[:])
                            nc.vector.tensor_copy(cagg[:, dsl], psb[:])
                        else:
                            nc.vector.tensor_add(
                                out=magg[:, dsl], in0=magg[:, dsl], in1=psa[:])
                            nc.vector.tensor_add(
                                out=cagg[:, dsl], in0=cagg[:, dsl], in1=psb[:])

            # ---- node MLP ----
            off = 0
            while off < NPAD:
                csz = min(CE, NPAD - off)
                sl = slice(off, off + csz)
                h1_ps = ps_big.tile([D, csz], dt.float32, tag="big")
                nc.tensor.matmul(h1_ps[:], W_n1a[:], nfT_loc[:, sl],
                                 start=True, stop=False)
                nc.tensor.matmul(h1_ps[:], W_n1b[:], magg[:, sl],
                                 start=False, stop=True)
                h1s = wpool.tile([D, csz], dt.float32, tag="h1s")
                nc.scalar.activation(h1s[:], h1_ps[:], AF.Silu, bias=b_n1[:])
                h2_ps = ps_big.tile([D, csz], dt.float32, tag="big")
                nc.tensor.matmul(h2_ps[:], W_n2[:], h1s[:], start=True, stop=True)
                hout = wpool.tile([D, csz], dt.float32, tag="hout")
                nc.scalar.activation(hout[:], h2_ps[:], AF.Identity, bias=b_n2[:])
                nc.vector.tensor_add(out=hout[:], in0=hout[:], in1=nfT_loc[:, sl])
                nc.sync.dma_start(houtT[:, sl], hout[:])
                off += csz

            # ---- coords epilogue ----
            nc.vector.tensor_add(out=cagg[:], in0=cagg[:], in1=coordsT_loc[:])
            nc.sync.dma_start(coutT[:], cagg[:])

    nc.compile()
    return nc


_cache = {}
LAST_RESULT = None


def kernel(**inputs):
    from concourse.bass_utils import run_bass_kernel_spmd

    shared, cores, meta = _prep(inputs)
    nc = _build_nc(meta)

    in_maps = []
    for k in range(NCORES):
        full = dict(cores[k])
        full.update(shared)
        in_maps.append(full)

    import os
    trace = bool(int(os.environ.get("KTRACE", "0")))
    res = run_bass_kernel_spmd(nc, in_maps, core_ids=list(range(NCORES)),
                               trace=trace)
    global LAST_RESULT
    LAST_RESULT = res
    houts, couts = [], []
    for k in range(NCORES):
        houts.append(np.asarray(res.results[k]["houtT"])[:, :NLOC].T)
        couts.append(np.asarray(res.results[k]["coutT"])[:, :NLOC].T)
    h_out = np.ascontiguousarray(np.concatenate(houts, 0), dtype=np.float32)
    coords_out = np.ascontiguousarray(np.concatenate(couts, 0), dtype=np.float32)
    return h_out, coords_out


# revision 12
# speedup vs baseline: 1.3982x; 1.3982x over previous
"""EGNN graph-conv (DGL EnGraphConv style) Trainium2 kernel, 8 NeuronCores.

Sharding: edges partitioned by dst node-range (edge/dst-parallel) — each core
owns 6250 nodes and all edges pointing into them, so the segment-sum needs no
cross-core reduction (outputs concatenate).

Device pipeline (per core, feature-major: features on partitions, edges on
the free axis):
  - node features gathered per edge endpoint with the SWDGE transpose
    dma_gather (bf16, 256B rows) — src from the replicated global table in two
    int16-index windows, dst from the core-local table,
  - edge MLP / attention gate / coord head as chained PE matmuls (f32 PSUM),
  - segment-sum scatter as a per-128-edge-group matmul against an on-chip
    0/1 selection matrix S (iota==dstrel), accumulated per 64-node bin in
    PSUM, evacuated to an SBUF accumulator,
  - node MLP + residual in f32, outputs written feature-major and
    transposed/concatenated on the host.

Host-side prep is index/layout work only (sort edges, build CSR-like bins,
int16 gather indices, per-edge geometric features radial/coord_diff, degree
counts) — all model FLOPs (MLPs, gate, scatter sums, residuals) run on
device.
"""
import numpy as np
import ml_dtypes

N = 50000
E = 800000
D = 128
DE = 32
NCORES = 8
NLOC = N // NCORES            # 6250
BINW = 64
NBINS = (NLOC + BINW - 1) // BINW   # 98
NPAD = NBINS * BINW           # 6272
WIN = 32768
BASE1 = N - WIN               # 17232
G = 128                       # scatter group (edges)
CE = 512                      # matmul chunk (edges)

bf16 = ml_dtypes.bfloat16
f32 = np.float32


def _bf(x):
    return np.asarray(x, dtype=bf16)


def _prep(inputs):
    src = np.asarray(inputs["src"])
    dst = np.asarray(inputs["dst"])
    nf = np.asarray(inputs["node_feats"], f32)
    coords = np.asarray(inputs["coords"], f32)
    ef = np.asarray(inputs["edge_feats"], f32)

    core_of = dst // NLOC
    deg = np.bincount(dst, minlength=N).astype(f32)

    per_core = []
    cnt = np.zeros((NCORES, 2, NBINS), np.int64)
    for k in range(NCORES):
        sel = np.nonzero(core_of == k)[0]
        s, d = src[sel], dst[sel]
        dloc = d - k * NLOC
        b = dloc // BINW
        w = (s >= WIN).astype(np.int64)
        order = np.lexsort((s, b, w))
        per_core.append((sel[order], s[order], dloc[order], b[order], w[order]))
        np.add.at(cnt[k], (w, b), 1)

    Gwb = np.maximum(1, -(-cnt.max(axis=0) // G))       # [2, NBINS]
    for wv in range(2):
        Gwb[wv, NBINS - 1] += (-Gwb[wv].sum()) % (CE // G)
    ngroups = int(Gwb.sum())
    EC = ngroups * G
    group_bin, group_win = [], []
    for wv in range(2):
        for bv in range(NBINS):
            group_bin += [bv] * int(Gwb[wv, bv])
            group_win += [wv] * int(Gwb[wv, bv])
    sw0 = int(Gwb[0].sum()) * G

    cores = []
    for k in range(NCORES):
        eidx, s, dloc, b, w = per_core[k]
        idx_src = np.zeros(EC, np.int16)
        idx_dst = np.zeros(EC, np.int16)
        efx = np.zeros((EC, DE + 1), f32)
        cd3 = np.zeros((EC, 3), f32)
        dstrel = np.full(EC, -1.0, f32)
        pos = 0
        ptr = 0
        for wv in range(2):
            for bv in range(NBINS):
                n = int(cnt[k, wv, bv])
                seg = slice(ptr, ptr + n)
                o = slice(pos, pos + n)
                e_ids = eidx[seg]
                idx_src[o] = (s[seg] - (BASE1 if wv else 0)).astype(np.int16)
                idx_dst[o] = dloc[seg].astype(np.int16)
                efx[o, :DE] = ef[e_ids]
                diff = coords[src[e_ids]] - coords[dst[e_ids]]
                w_ic = 1.0 / np.maximum(deg[dst[e_ids]], 1.0)
                cd3[o] = diff * w_ic[:, None]
                efx[o, DE] = (diff * diff).sum(-1)
                dstrel[o] = (dloc[seg] - bv * BINW).astype(f32)
                ptr += n
                pos += int(Gwb[wv, bv]) * G

        nf16_loc = np.zeros((NPAD, D), bf16)
        nf16_loc[:NLOC] = _bf(nf[k * NLOC:(k + 1) * NLOC])
        nfT_loc = np.zeros((D, NPAD), f32)
        nfT_loc[:, :NLOC] = nf[k * NLOC:(k + 1) * NLOC].T
        coordsT_loc = np.zeros((3, NPAD), f32)
        coordsT_loc[:, :NLOC] = coords[k * NLOC:(k + 1) * NLOC].T
        cores.append(dict(
            idx_src=np.tile(idx_src.reshape(-1, 16).T, (8, 1)).copy(),
            idx_dst=np.tile(idx_dst.reshape(-1, 16).T, (8, 1)).copy(),
            efT=_bf(efx.T).copy(),
            cd3=np.ascontiguousarray(
                cd3.reshape(ngroups, G, 3).transpose(1, 0, 2).reshape(G, -1)),
            dstrel=np.ascontiguousarray(dstrel.reshape(ngroups, G).T),
            nf16_loc=nf16_loc, nfT_loc=nfT_loc,
            coordsT_loc=coordsT_loc,
        ))

    shared = dict(
        nf16_glob=_bf(nf),
        iota=np.broadcast_to(np.arange(BINW, dtype=f32), (G, BINW)).copy(),
        ident_ext=np.concatenate(
            [np.eye(D, dtype=f32), np.asarray(inputs["W_a"], f32)], 1).astype(bf16),
        W_e1a=_bf(inputs["W_e1"][:D]), W_e1b=_bf(inputs["W_e1"][D:2 * D]),
        W_e1c=_bf(np.concatenate(
            [inputs["W_e1"][2 * D + 1:], inputs["W_e1"][2 * D:2 * D + 1]], 0)),
        W_e2=_bf(inputs["W_e2"]), W_c1=_bf(inputs["W_c1"]), W_c=_bf(inputs["W_c"]),
        W_n1a=np.asarray(inputs["W_n1"][:D], f32),
        W_n1b=np.asarray(inputs["W_n1"][D:], f32),
        W_n2=np.asarray(inputs["W_n2"], f32),
        b_e1=np.asarray(inputs["b_e1"], f32).reshape(D, 1),
        b_e2=np.asarray(inputs["b_e2"], f32).reshape(D, 1),
        b_c1=np.asarray(inputs["b_c1"], f32).reshape(D, 1),
        b_n1=np.asarray(inputs["b_n1"], f32).reshape(D, 1),
        b_n2=np.asarray(inputs["b_n2"], f32).reshape(D, 1),
    )
    meta = dict(EC=EC, ngroups=ngroups, group_bin=group_bin,
                group_win=group_win, sw0=sw0,
                b_a=float(np.asarray(inputs["b_a"]).reshape(())))
    return shared, cores, meta


def _build_nc(meta):
    import concourse.bacc as bacc
    import concourse.mybir as mybir
    import concourse.tile as tile
    from concourse import library_config

    EC, ngroups = meta["EC"], meta["ngroups"]
    gb, gw, sw0 = meta["group_bin"], meta["group_win"], meta["sw0"]
    b_a = meta["b_a"]
    dt = mybir.dt
    AF = mybir.ActivationFunctionType
    OP = mybir.AluOpType

    nc = bacc.Bacc("TRN2", target_bir_lowering=False)

    # dram I/O
    din = {}
    def I(name, shape, dtype):
        din[name] = nc.dram_tensor(name, shape, dtype, kind="ExternalInput")
        return din[name]

    I("idx_src", [128, EC // 16], dt.int16)
    I("idx_dst", [128, EC // 16], dt.int16)
    I("efT", [DE + 1, EC], dt.bfloat16)
    I("cd3", [G, ngroups * 3], dt.float32)
    I("dstrel", [G, ngroups], dt.float32)
    I("nf16_loc", [NPAD, D], dt.bfloat16)
    I("nfT_loc", [D, NPAD], dt.float32)
    I("coordsT_loc", [3, NPAD], dt.float32)
    I("nf16_glob", [N, D], dt.bfloat16)
    I("iota", [G, BINW], dt.float32)
    I("ident_ext", [D, D + 1], dt.bfloat16)
    I("W_e1a", [D, D], dt.bfloat16)
    I("W_e1b", [D, D], dt.bfloat16)
    I("W_e1c", [DE + 1, D], dt.bfloat16)
    I("W_e2", [D, D], dt.bfloat16)
    I("W_c1", [D, D], dt.bfloat16)
    I("W_c", [D, 1], dt.bfloat16)
    I("W_n1a", [D, D], dt.float32)
    I("W_n1b", [D, D], dt.float32)
    I("W_n2", [D, D], dt.float32)
    for bname in ["b_e1", "b_e2", "b_c1", "b_n1", "b_n2"]:
        I(bname, [D, 1], dt.float32)

    houtT = nc.dram_tensor("houtT", [D, NPAD], dt.float32, kind="ExternalOutput")
    coutT = nc.dram_tensor("coutT", [3, NPAD], dt.float32, kind="ExternalOutput")

    nchunks = EC // CE

    with tile.TileContext(nc) as tc:
        nc.gpsimd.load_library(library_config.mlp)
        with (
            tc.tile_pool(name="const", bufs=1) as cpool,
            tc.tile_pool(name="gath", bufs=3) as gpool,
            tc.tile_pool(name="work", bufs=3) as wpool,
            tc.tile_pool(name="grp", bufs=4) as grpool,
            tc.tile_pool(name="acc", bufs=1) as apool,
            tc.tile_pool(name="pbig", bufs=3, space="PSUM") as ps_big,
            tc.tile_pool(name="ptg", bufs=2, space="PSUM") as ps_tg,
            tc.tile_pool(name="psc", bufs=1, space="PSUM") as ps_sc,
        ):
            # ---- constants to SBUF ----
            def load_const(name, shape, dtype):
                t = cpool.tile(shape, dtype, tag=f"c_{name}")
                nc.sync.dma_start(t[:], din[name][:])
                return t

            idx_src = load_const("idx_src", [128, EC // 16], dt.int16)
            idx_dst = load_const("idx_dst", [128, EC // 16], dt.int16)
            cd3 = load_const("cd3", [G, ngroups * 3], dt.float32)
            dstrel = load_const("dstrel", [G, ngroups], dt.float32)
            nfT_loc = load_const("nfT_loc", [D, NPAD], dt.float32)
            coordsT_loc = load_const("coordsT_loc", [3, NPAD], dt.float32)
            iota = load_const("iota", [G, BINW], dt.float32)
            ident_ext = load_const("ident_ext", [D, D + 1], dt.bfloat16)
            W_e1a = load_const("W_e1a", [D, D], dt.bfloat16)
            W_e1b = load_const("W_e1b", [D, D], dt.bfloat16)
            W_e1c = load_const("W_e1c", [DE + 1, D], dt.bfloat16)
            W_e2 = load_const("W_e2", [D, D], dt.bfloat16)
            W_c1 = load_const("W_c1", [D, D], dt.bfloat16)
            W_c = load_const("W_c", [D, 1], dt.bfloat16)
            W_n1a = load_const("W_n1a", [D, D], dt.float32)
            W_n1b = load_const("W_n1b", [D, D], dt.float32)
            W_n2 = load_const("W_n2", [D, D], dt.float32)
            b_e1 = load_const("b_e1", [D, 1], dt.float32)
            b_e2 = load_const("b_e2", [D, 1], dt.float32)
            b_c1 = load_const("b_c1", [D, 1], dt.float32)
            b_n1 = load_const("b_n1", [D, 1], dt.float32)
            b_n2 = load_const("b_n2", [D, 1], dt.float32)

            magg = apool.tile([D, NPAD], dt.float32)     # segment-sum accum
            cagg = apool.tile([3, NPAD], dt.float32)

            src_windows = [din["nf16_glob"][0:WIN, :],
                           din["nf16_glob"][BASE1:BASE1 + WIN, :]]

            psa = psb = None
            for c in range(nchunks):
                e0 = c * CE
                w = gw[c * (CE // G)]
                assert all(gw[c * (CE // G) + j] == w for j in range(CE // G))

                src_t = gpool.tile([128, 1, CE], dt.bfloat16, tag="src")
                nc.gpsimd.dma_gather(
                    src_t[:], src_windows[w], idx_src[:, e0 // 16:(e0 + CE) // 16],
                    CE, CE, D, transpose=True)
                dst_t = gpool.tile([128, 1, CE], dt.bfloat16, tag="dst")
                nc.gpsimd.dma_gather(
                    dst_t[:], din["nf16_loc"][:], idx_dst[:, e0 // 16:(e0 + CE) // 16],
                    CE, CE, D, transpose=True)
                ef_t = gpool.tile([DE + 1, CE], dt.bfloat16, tag="ef")
                nc.sync.dma_start(ef_t[:], din["efT"][:, e0:e0 + CE])

                m1_ps = ps_big.tile([D, CE], dt.float32, tag="big")
                nc.tensor.matmul(m1_ps[:], W_e1a[:], src_t[:, 0, :],
                                 start=True, stop=False)
                nc.tensor.matmul(m1_ps[:], W_e1b[:], dst_t[:, 0, :],
                                 start=False, stop=False)
                nc.tensor.matmul(m1_ps[:], W_e1c[:], ef_t[:],
                                 start=False, stop=True)
                m1s = wpool.tile([D, CE], dt.bfloat16, tag="m1s")
                nc.scalar.activation(m1s[:], m1_ps[:], AF.Silu, bias=b_e1[:])

                m2_ps = ps_big.tile([D, CE], dt.float32, tag="big")
                nc.tensor.matmul(m2_ps[:], W_e2[:], m1s[:], start=True, stop=True)
                mT = wpool.tile([D, CE], dt.bfloat16, tag="mT")
                nc.scalar.activation(mT[:], m2_ps[:], AF.Silu, bias=b_e2[:])

                c1_ps = ps_big.tile([D, CE], dt.float32, tag="big")
                nc.tensor.matmul(c1_ps[:], W_c1[:], mT[:], start=True, stop=True)
                c1T = wpool.tile([D, CE], dt.bfloat16, tag="c1T")
                nc.scalar.activation(c1T[:], c1_ps[:], AF.Silu, bias=b_c1[:])

                for j in range(CE // G):
                    g = c * (CE // G) + j
                    gs = slice(j * G, (j + 1) * G)
                    b0 = gb[g] * BINW

                    tg_ps = ps_tg.tile([G, D + 1], dt.float32, tag="tg")
                    nc.tensor.matmul(tg_ps[:], mT[:, gs], ident_ext[:],
                                     start=True, stop=True)
                    siga = grpool.tile([G, 1], dt.float32, tag="siga")
                    nc.scalar.activation(siga[:], tg_ps[:, D:D + 1],
                                         AF.Sigmoid, bias=b_a)
                    c_ps = ps_sc.tile([G, 1], dt.float32, tag="cps")
                    nc.tensor.matmul(c_ps[:], c1T[:, gs], W_c[:],
                                     start=True, stop=True)
                    c_sb = grpool.tile([G, 1], dt.float32, tag="csb")
                    nc.vector.tensor_copy(c_sb[:], c_ps[:])

                    pm = grpool.tile([G, D], dt.bfloat16, tag="pm")
                    nc.vector.tensor_scalar_mul(pm[:], tg_ps[:, 0:D], siga[:])
                    pay3 = grpool.tile([G, 3], dt.bfloat16, tag="pay3")
                    nc.vector.tensor_scalar_mul(
                        pay3[:], cd3[:, g * 3:(g + 1) * 3], c_sb[:])
                    S = grpool.tile([G, BINW], dt.bfloat16, tag="S")
                    nc.vector.tensor_scalar(
                        S[:], iota[:], dstrel[:, g:g + 1], None, op0=OP.is_equal)

                    first = (g == 0) or (gb[g - 1] != gb[g]) or (gw[g - 1] != gw[g])
                    last = (g == ngroups - 1) or (gb[g + 1] != gb[g]) \
                        or (gw[g + 1] != gw[g])
                    if first:
                        psa = ps_sc.tile([D, BINW], dt.float32, tag="psa")
                        psb = ps_sc.tile([3, BINW], dt.float32, tag="psb")
                    nc.tensor.matmul(psa[:], pm[:], S[:], start=first, stop=last)
                    nc.tensor.matmul(psb[:], pay3[:], S[:], start=first, stop=last)
                    if last:
                        dsl = slice(b0, b0 + BINW)
                        if gw[g] == 0:
                            nc.vector.tensor_copy(magg[:, dsl], psa[:])
                            nc.vector.tensor_copy(cagg[:, dsl], psb[:])
                        else:
                            nc.vector.tensor_add(
                                out=magg[:, dsl], in0=magg[:, dsl], in1=psa[:])
                            nc.vector.tensor_add(
                                out=cagg[:, dsl], in0=cagg[:, dsl], in1=psb[:])

            # ---- node MLP ----
            off = 0
            while off < NPAD:
                csz = min(CE, NPAD - off)
                sl = slice(off, off + csz)
                h1_ps = ps_big.tile([D, csz], dt.float32, tag="big")
                nc.tensor.matmul(h1_ps[:], W_n1a[:], nfT_loc[:, sl],
                                 start=True, stop=False)
                nc.tensor.matmul(h1_ps[:], W_n1b[:], magg[:, sl],
                                 start=False, stop=True)
                h1s = wpool.tile([D, csz], dt.float32, tag="h1s")
                nc.scalar.activation(h1s[:], h1_ps[:], AF.Silu, bias=b_n1[:])
                h2_ps = ps_big.tile([D, csz], dt.float32, tag="big")
                nc.tensor.matmul(h2_ps[:], W_n2[:], h1s[:], start=True, stop=True)
                hout = wpool.tile([D, csz], dt.float32, tag="hout")
                nc.scalar.activation(hout[:], h2_ps[:], AF.Identity, bias=b_n2[:])
                nc.vector.tensor_add(out=hout[:], in0=hout[:], in1=nfT_loc[:, sl])
                nc.sync.dma_start(houtT[:, sl], hout[:])
                off += csz

            # ---- coords epilogue ----
            nc.vector.tensor_add(out=cagg[:], in0=cagg[:], in1=coordsT_loc[:])
            nc.sync.dma_start(coutT[:], cagg[:])

    nc.compile()
    return nc


_cache = {}
LAST_RESULT = None


def kernel(**inputs):
    from concourse.bass_utils import run_bass_kernel_spmd

    shared, cores, meta = _prep(inputs)
    nc = _build_nc(meta)

    in_maps = []
    for k in range(NCORES):
        full = dict(cores[k])
        full.update(shared)
        in_maps.append(full)

    import os
    trace = bool(int(os.environ.get("KTRACE", "0")))
    res = run_bass_kernel_spmd(nc, in_maps, core_ids=list(range(NCORES)),
                               trace=trace)
    global LAST_RESULT
    LAST_RESULT = res
    houts, couts = [], []
    for k in range(NCORES):
        houts.append(np.asarray(res.results[k]["houtT"])[:, :NLOC].T)
        couts.append(np.asarray(res.results[k]["coutT"])[:, :NLOC].T)
    h_out = np.ascontiguousarray(np.concatenate(houts, 0), dtype=np.float32)
    coords_out = np.ascontiguousarray(np.concatenate(couts, 0), dtype=np.float32)
    return h_out, coords_out
